# revision 9
# baseline (speedup 1.0000x reference)
"""Trainium2 Bass kernel for nn_CustomAttentionLayer (single-'head' attention
over the full 2048 hidden dim, with module-level RoPE).

Sharding: sequence-parallel over 8 NeuronCores. Each core computes the
q/k/v projections + RoPE for its S/8 = 512 sequence rows (both batches),
exchanges the k_rot/v shards with on-device AllGathers, then runs attention
plus the output projection for its own 512 query rows. The host concatenates
the per-core output shards. The (shared) weights are uploaded sharded 1/8th
per core and broadcast on-device with an AllGather to keep the host->device
transfer small.

Precision: everything runs in float32r (rounded fp32; full PE throughput for
moving dims >= 256) with fp32 PSUM accumulation. Softmax runs unnormalized
(exp without max subtraction -- the fp32 exponent range covers the logit
range) and the per-row normalization is folded in after the output
projection.
"""
import sys
sys.path.insert(0, "/opt/trn_rl_repo")

import numpy as np

from concourse import bacc
import concourse.mybir as mybir
import concourse.tile as tile
from concourse.bass_utils import run_bass_kernel_spmd
from concourse.masks import make_identity

B, S, H = 2, 4096, 2048
NC_ = 8
SS = S // NC_          # 512 sequence rows per core
C = B * SS             # 1024 columns per core (b-major)
D2 = H // 2
SCALE = 1.0 / 8.0
HCH = H // 128         # 16 hidden chunks
PAIRS = D2 // 128      # 8 rope pairs

F32 = mybir.dt.float32
F32R = mybir.dt.float32r

_NC_CACHE = {}


def build_kernel():
    nc = bacc.Bacc("TRN2", target_bir_lowering=False, debug=False, num_devices=NC_)

    # ---- per-core I/O ----
    hid_t = nc.dram_tensor("hid_t", [H, C], F32R, kind="ExternalInput")
    # 1/8 slice of vstack(wq_t, wk_t, wv_t, wo_t) [4H/8 = 1024, H]
    w_sl = nc.dram_tensor("w_sl", [4 * H // NC_, H], F32R, kind="ExternalInput")
    cos_s = nc.dram_tensor("cos_s", [D2, SS], F32, kind="ExternalInput")
    sin_s = nc.dram_tensor("sin_s", [D2, SS], F32, kind="ExternalInput")

    out_o = nc.dram_tensor("out_o", [C, H], F32, kind="ExternalOutput")
    krot_o = nc.dram_tensor("krot_o", [C, H], F32, kind="ExternalOutput")
    v_o = nc.dram_tensor("v_o", [C, H], F32R, kind="ExternalOutput")

    # ---- internal DRAM ----
    w_bounce = nc.dram_tensor("w_bounce", [4 * H // NC_, H], F32R)
    w_ag = nc.dram_tensor("w_ag", [4 * H, H], F32R, addr_space="Shared")
    wq_t, wk_t, wv_t, wo_t = (w_ag[i * H:(i + 1) * H, :] for i in range(4))
    k_ag_in = nc.dram_tensor("k_ag_in", [H, C], F32R)
    k_ag = nc.dram_tensor("k_ag", [NC_ * H, C], F32R, addr_space="Shared")
    v_ag_in = nc.dram_tensor("v_ag_in", [C, H], F32R)
    v_ag = nc.dram_tensor("v_ag", [NC_ * C, H], F32R, addr_space="Shared")
    qrot_d = nc.dram_tensor("qrot_d", [H, C], F32R)

    hid_v = hid_t.rearrange("(c p) n -> p c n", p=128)    # [128, 16, C]

    with tile.TileContext(nc) as tc:
        # broadcast the weights before anything else
        nc.sync.dma_start(w_bounce[:], w_sl[:])
        nc.gpsimd.collective_compute(
            "AllGather", mybir.AluOpType.bypass,
            ins=[w_bounce[:]], outs=[w_ag[:]],
            replica_groups=[list(range(NC_))],
        )

        with tc.tile_pool(name="const", bufs=1) as constp:
            iden32 = constp.tile([128, 128], F32)
            make_identity(nc, iden32[:])
            iden_r = constp.tile([128, 128], F32R)
            nc.vector.tensor_copy(iden_r[:], iden32[:])
            iden1 = constp.tile([1, 1], F32)
            nc.vector.memset(iden1[:], 1.0)
            ones32 = constp.tile([128, 1], F32)
            nc.vector.memset(ones32[:], 1.0)
            ones_r = constp.tile([128, 1], F32R)
            nc.vector.tensor_copy(ones_r[:], ones32[:])

            with tc.tile_pool(name="big", bufs=1) as bigp:
                hid_sb = bigp.tile([128, HCH, C], F32R)       # 8 MB, all phases
                nc.sync.dma_start(hid_sb[:], hid_v)

                def projection_phase(w_dram, which):
                    """K or Q: project, rope, write k_ag_in/qrot_d (+ krot_o for K)."""
                    with (
                        tc.tile_pool(name=f"wblk_{which}", bufs=3) as wblkp,
                        tc.tile_pool(name=f"cosj_{which}", bufs=3) as cosjp,
                        tc.tile_pool(name=f"kt_{which}", bufs=4) as ktp,
                        tc.tile_pool(name=f"rope_{which}", bufs=2) as ropep,
                        tc.tile_pool(name=f"krot_{which}", bufs=2) as krotp,
                        tc.tile_pool(name=f"ps_{which}", bufs=4, space="PSUM") as psp,
                        tc.tile_pool(name=f"pstr_{which}", bufs=2, space="PSUM") as pstr,
                        tc.tile_pool(name=f"knat_{which}", bufs=3) as knatp,
                    ):
                        dst = k_ag_in if which == "k" else qrot_d
                        for j in range(PAIRS):
                            cj = cosjp.tile([128, C], F32, name="cj", tag="cj")
                            sj = cosjp.tile([128, C], F32, name="sj", tag="sj")
                            for half in range(B):
                                nc.sync.dma_start(
                                    cj[:, half * SS:(half + 1) * SS],
                                    cos_s[j * 128:(j + 1) * 128, :])
                                nc.sync.dma_start(
                                    sj[:, half * SS:(half + 1) * SS],
                                    sin_s[j * 128:(j + 1) * 128, :])
                            raws = []
                            for part in (j, j + PAIRS):
                                wb = wblkp.tile([128, HCH, 128], F32R, name="wb", tag="wb")
                                w_view = w_dram[:, part * 128:(part + 1) * 128].rearrange(
                                    "(c p) m -> p c m", p=128)
                                nc.sync.dma_start(wb[:], w_view)
                                raw = ktp.tile([128, C], F32, name="raw", tag="raw")
                                for nchk in range(C // 512):
                                    ps = psp.tile([128, 512], F32, name="ps", tag="ps")
                                    for hch in range(HCH):
                                        nc.tensor.matmul(
                                            ps[:], wb[:, hch, :],
                                            hid_sb[:, hch, nchk * 512:(nchk + 1) * 512],
                                            start=(hch == 0), stop=(hch == HCH - 1),
                                        )
                                    nc.scalar.copy(raw[:, nchk * 512:(nchk + 1) * 512], ps[:])
                                raws.append(raw)
                            re, im = raws
                            t1 = ropep.tile([128, C], F32, name="t1", tag="t1")
                            t2 = ropep.tile([128, C], F32, name="t2", tag="t2")
                            rot_re = krotp.tile([128, C], F32R, name="rot_re", tag="rot_re")
                            rot_im = krotp.tile([128, C], F32R, name="rot_im", tag="rot_im")
                            nc.vector.tensor_mul(t1[:], re[:], cj[:])
                            nc.vector.tensor_mul(t2[:], im[:], sj[:])
                            nc.vector.tensor_tensor(rot_re[:], t1[:], t2[:],
                                                    mybir.AluOpType.subtract)
                            nc.vector.tensor_mul(t1[:], re[:], sj[:])
                            nc.vector.tensor_mul(t2[:], im[:], cj[:])
                            nc.vector.tensor_tensor(rot_im[:], t1[:], t2[:],
                                                    mybir.AluOpType.add)
                            nc.sync.dma_start(dst[j * 128:(j + 1) * 128, :], rot_re[:])
                            nc.sync.dma_start(dst[D2 + j * 128:D2 + (j + 1) * 128, :],
                                              rot_im[:])
                            if which == "k":
                                # natural interleaved k_rot output
                                for sch in range(C // 128):
                                    mini = knatp.tile([128, 256], F32, name="mini", tag="mini")
                                    tpr = pstr.tile([128, 128], F32R, name="tpr", tag="tpr")
                                    nc.tensor.transpose(
                                        tpr[:], rot_re[:, sch * 128:(sch + 1) * 128], iden_r[:])
                                    nc.scalar.copy(mini[:, 0::2], tpr[:])
                                    tpi = pstr.tile([128, 128], F32R, name="tpi", tag="tpi")
                                    nc.tensor.transpose(
                                        tpi[:], rot_im[:, sch * 128:(sch + 1) * 128], iden_r[:])
                                    nc.scalar.copy(mini[:, 1::2], tpi[:])
                                    nc.sync.dma_start(
                                        krot_o[sch * 128:(sch + 1) * 128,
                                               256 * j:256 * (j + 1)],
                                        mini[:])

                projection_phase(wk_t, "k")

                # ---------------- V projection ----------------
                OG_V = 256
                with (
                    tc.tile_pool(name="vblk", bufs=2) as vblkp,
                    tc.tile_pool(name="v32", bufs=1) as v32p,
                    tc.tile_pool(name="ps_v", bufs=4, space="PSUM") as psvp,
                ):
                    v32s = [v32p.tile([128, H], F32R, name=f"v32_{sch}", tag=f"v32_{sch}")
                            for sch in range(C // 128)]
                    for og in range(H // OG_V):
                        vb = vblkp.tile([128, HCH, OG_V], F32R, name="vb", tag="vb")
                        nc.sync.dma_start(
                            vb[:], wv_t[:, og * OG_V:(og + 1) * OG_V].rearrange(
                                "(c p) m -> p c m", p=128))
                        for sch in range(C // 128):
                            ps = psvp.tile([128, OG_V], F32, name="psv", tag="psv")
                            for hch in range(HCH):
                                nc.tensor.matmul(
                                    ps[:], hid_sb[:, hch, sch * 128:(sch + 1) * 128],
                                    vb[:, hch, :],
                                    start=(hch == 0), stop=(hch == HCH - 1),
                                )
                            nc.scalar.copy(v32s[sch][:, og * OG_V:(og + 1) * OG_V], ps[:])
                    for sch in range(C // 128):
                        nc.sync.dma_start(v_ag_in[sch * 128:(sch + 1) * 128, :], v32s[sch][:])
                        nc.sync.dma_start(v_o[sch * 128:(sch + 1) * 128, :], v32s[sch][:])

                # ---------------- k/v collectives ----------------
                nc.gpsimd.collective_compute(
                    "AllGather", mybir.AluOpType.bypass,
                    ins=[k_ag_in[:]], outs=[k_ag[:]],
                    replica_groups=[list(range(NC_))],
                )
                nc.gpsimd.collective_compute(
                    "AllGather", mybir.AluOpType.bypass,
                    ins=[v_ag_in[:]], outs=[v_ag[:]],
                    replica_groups=[list(range(NC_))],
                )

                projection_phase(wq_t, "q")

            # ---------------- attention ----------------
            KC = S // 128              # 32 context chunks per batch
            with (
                tc.tile_pool(name="qb", bufs=1) as qbp,
                tc.tile_pool(name="kslab", bufs=3) as kslabp,
                tc.tile_pool(name="exps", bufs=1) as expp,
                tc.tile_pool(name="vslab", bufs=4) as vslabp,
                tc.tile_pool(name="ctx", bufs=1) as ctxp,
                tc.tile_pool(name="woblk", bufs=2) as wop,
                tc.tile_pool(name="outs", bufs=4) as outp,
                tc.tile_pool(name="den", bufs=1) as denp,
                tc.tile_pool(name="pss", bufs=2, space="PSUM") as pss,
                tc.tile_pool(name="psden", bufs=1, space="PSUM") as psden,
                tc.tile_pool(name="psctx", bufs=1, space="PSUM") as psctx,
                tc.tile_pool(name="psout", bufs=2, space="PSUM") as psout,
                tc.tile_pool(name="pstrd", bufs=1, space="PSUM") as pstrd,
            ):
                for b in range(B):
                    qb = qbp.tile([128, HCH, 512], F32R, name="qb", tag="qb")
                    nc.sync.dma_start(
                        qb[:],
                        qrot_d[:, b * 512:(b + 1) * 512].rearrange(
                            "(c p) q -> p c q", p=128))

                    exp_tiles = []
                    den_ps = psden.tile([1, 512], F32, name="den_ps", tag="den_ps")
                    for kc in range(KC):
                        r, l = kc // 4, kc % 4
                        kslab = kslabp.tile([128, HCH, 128], F32R, name="kslab", tag="kslab")
                        k_view = k_ag[r * H:(r + 1) * H,
                                      b * 512 + l * 128: b * 512 + (l + 1) * 128]
                        nc.sync.dma_start(
                            kslab[:], k_view.rearrange("(c p) n -> p c n", p=128))
                        ps_s = pss.tile([128, 512], F32, name="ps_s", tag="ps_s")
                        for hch in range(HCH):
                            nc.tensor.matmul(
                                ps_s[:], kslab[:, hch, :], qb[:, hch, :],
                                start=(hch == 0), stop=(hch == HCH - 1),
                            )
                        et = expp.tile([128, 512], F32R, name=f"exp{kc}", tag=f"exp{kc}")
                        nc.scalar.activation(et[:], ps_s[:],
                                             mybir.ActivationFunctionType.Exp,
                                             bias=0.0, scale=SCALE)
                        exp_tiles.append(et)
                        nc.tensor.matmul(den_ps[:], ones_r[:], et[:],
                                         start=(kc == 0), stop=(kc == KC - 1))

                    # denominators -> per-q-row reciprocals [128, 4]
                    den_row = denp.tile([1, 512], F32, name="den_row", tag="den_row")
                    nc.scalar.copy(den_row[:], den_ps[:])
                    den_col = denp.tile([128, 4], F32, name="den_col", tag="den_col")
                    for qs in range(4):
                        tp = pstrd.tile([128, 1], F32, name="tpd", tag="tpd")
                        nc.tensor.transpose(tp[:], den_row[:, qs * 128:(qs + 1) * 128],
                                            iden1[:])
                        nc.scalar.copy(den_col[:, qs:qs + 1], tp[:])
                    recip = denp.tile([128, 4], F32, name="recip", tag="recip")
                    nc.vector.reciprocal(recip[:], den_col[:])

                    # ctx_t[o, q] = sum_k v[k, o] * numer[k, q]
                    OG_C = 256
                    ctx_tiles = []
                    for og in range(H // OG_C):
                        ps_c = [psctx.tile([128, 512], F32, name=f"psc{os_}", tag=f"psc{os_}")
                                for os_ in range(OG_C // 128)]
                        for kc in range(KC):
                            r, l = kc // 4, kc % 4
                            vslab = vslabp.tile([128, OG_C], F32R, name="vslab", tag="vslab")
                            nc.sync.dma_start(
                                vslab[:],
                                v_ag[r * C + b * 512 + l * 128:
                                     r * C + b * 512 + (l + 1) * 128,
                                     og * OG_C:(og + 1) * OG_C])
                            for os_ in range(OG_C // 128):
                                nc.tensor.matmul(
                                    ps_c[os_][:], vslab[:, os_ * 128:(os_ + 1) * 128],
                                    exp_tiles[kc][:],
                                    start=(kc == 0), stop=(kc == KC - 1),
                                )
                        for os_ in range(OG_C // 128):
                            oc = og * (OG_C // 128) + os_
                            ct = ctxp.tile([128, 512], F32R, name=f"ctx{oc}", tag=f"ctx{oc}")
                            nc.scalar.copy(ct[:], ps_c[os_][:])
                            ctx_tiles.append(ct)

                    # out[q, o'] = (ctx_t.T @ wo_t) * recip[q]
                    OG_O = 256
                    for ogr in range(H // OG_O):
                        wob = wop.tile([128, HCH, OG_O], F32R, name="wob", tag="wob")
                        nc.sync.dma_start(
                            wob[:],
                            wo_t[:, ogr * OG_O:(ogr + 1) * OG_O].rearrange(
                                "(c p) m -> p c m", p=128))
                        for qs in range(4):
                            ps_o = psout.tile([128, OG_O], F32, name="ps_o", tag="ps_o")
                            for oc in range(HCH):
                                nc.tensor.matmul(
                                    ps_o[:], ctx_tiles[oc][:, qs * 128:(qs + 1) * 128],
                                    wob[:, oc, :],
                                    start=(oc == 0), stop=(oc == HCH - 1),
                                )
                            ot = outp.tile([128, OG_O], F32, name="ot", tag="ot")
                            nc.vector.tensor_scalar_mul(ot[:], ps_o[:], recip[:, qs:qs + 1])
                            nc.sync.dma_start(
                                out_o[b * 512 + qs * 128: b * 512 + (qs + 1) * 128,
                                      ogr * OG_O:(ogr + 1) * OG_O],
                                ot[:])

    nc.compile()
    return nc


def _get_nc():
    if "nc" not in _NC_CACHE:
        _NC_CACHE["nc"] = build_kernel()
    return _NC_CACHE["nc"]


def kernel(hidden_states, wq, wk, wv, wo, freqs_cos, freqs_sin, position_ids):
    hidden_states = np.asarray(hidden_states, dtype=np.float32)
    wq = np.asarray(wq, dtype=np.float32)
    wk = np.asarray(wk, dtype=np.float32)
    wv = np.asarray(wv, dtype=np.float32)
    wo = np.asarray(wo, dtype=np.float32)
    pos = np.asarray(position_ids)
    cos = np.asarray(freqs_cos, dtype=np.float32)[pos]   # [S, D2]
    sin = np.asarray(freqs_sin, dtype=np.float32)[pos]

    # vstack of the four transposed weights, sharded row-wise across cores
    w_all = np.concatenate([wq.T, wk.T, wv.T, wo.T], axis=0)
    w_all = np.ascontiguousarray(w_all)
    WS = 4 * H // NC_

    in_maps = []
    for i in range(NC_):
        sl = slice(i * SS, (i + 1) * SS)
        hid_i = np.ascontiguousarray(
            hidden_states[:, sl, :].transpose(2, 0, 1).reshape(H, C))
        in_maps.append({
            "hid_t": hid_i,
            "w_sl": np.ascontiguousarray(w_all[i * WS:(i + 1) * WS]),
            "cos_s": np.ascontiguousarray(cos[sl].T),
            "sin_s": np.ascontiguousarray(sin[sl].T),
        })

    nc = _get_nc()
    results = run_bass_kernel_spmd(nc, in_maps, list(range(NC_))).results

    out = np.empty((B, S, H), dtype=np.float32)
    k_rot = np.empty((B, S, H), dtype=np.float32)
    v = np.empty((B, S, H), dtype=np.float32)
    for i in range(NC_):
        sl = slice(i * SS, (i + 1) * SS)
        r = results[i]
        out[:, sl, :] = r["out_o"].reshape(B, SS, H)
        k_rot[:, sl, :] = r["krot_o"].reshape(B, SS, H)
        v[:, sl, :] = r["v_o"].reshape(B, SS, H)
    return out, k_rot, v


# revision 12
# speedup vs baseline: 1.0427x; 1.0427x over previous
"""Trainium2 Bass kernel for nn_CustomAttentionLayer (single-'head' attention
over the full 2048 hidden dim, with module-level RoPE).

Sharding: sequence-parallel over 8 NeuronCores. Each core computes the
q/k/v projections + RoPE for its S/8 = 512 sequence rows (both batches),
exchanges the k_rot/v shards with on-device AllGathers, then runs attention
plus the output projection for its own 512 query rows. The host concatenates
the per-core output shards. The (shared) weights are uploaded sharded 1/8th
per core and broadcast on-device with an AllGather; all weight blocks are
pre-swizzled on the host into [partition, h-chunk, cols] order so each SBUF
weight-tile load is one contiguous 8-16KB descriptor per partition.

Precision: everything runs in float32r (rounded fp32; full PE throughput for
moving dims >= 256) with fp32 PSUM accumulation. Softmax runs unnormalized
(exp without max subtraction -- the fp32 exponent range covers the logit
range) and the per-row normalization is folded in after the output
projection.
"""
import sys
sys.path.insert(0, "/opt/trn_rl_repo")

import numpy as np

from concourse import bacc
import concourse.mybir as mybir
import concourse.tile as tile
from concourse.bass_utils import run_bass_kernel_spmd
from concourse.masks import make_identity

B, S, H = 2, 4096, 2048
NC_ = 8
SS = S // NC_          # 512 sequence rows per core
C = B * SS             # 1024 columns per core (b-major)
D2 = H // 2
SCALE = 1.0 / 8.0
HCH = H // 128         # 16 hidden chunks
PAIRS = D2 // 128      # 8 rope pairs
WS = 4 * H // NC_      # weight-slice rows per core

F32 = mybir.dt.float32
F32R = mybir.dt.float32r

_NC_CACHE = {}


def build_kernel():
    nc = bacc.Bacc("TRN2", target_bir_lowering=False, debug=False, num_devices=NC_)

    # ---- per-core I/O (hid/w/cos pre-swizzled on host, see kernel()) ----
    hid_t = nc.dram_tensor("hid_t", [H, C], F32R, kind="ExternalInput")
    w_sl = nc.dram_tensor("w_sl", [WS, H], F32R, kind="ExternalInput")
    cos_s = nc.dram_tensor("cos_s", [D2, SS], F32, kind="ExternalInput")
    sin_s = nc.dram_tensor("sin_s", [D2, SS], F32, kind="ExternalInput")

    out_o = nc.dram_tensor("out_o", [C, H], F32, kind="ExternalOutput")
    krot_o = nc.dram_tensor("krot_o", [C, H], F32, kind="ExternalOutput")
    v_o = nc.dram_tensor("v_o", [C, H], F32R, kind="ExternalOutput")

    # ---- internal DRAM ----
    w_bounce = nc.dram_tensor("w_bounce", [WS, H], F32R)
    w_ag = nc.dram_tensor("w_ag", [4 * H, H], F32R, addr_space="Shared")
    k_ag_in = nc.dram_tensor("k_ag_in", [H, C], F32R)
    k_ag = nc.dram_tensor("k_ag", [NC_ * H, C], F32R, addr_space="Shared")
    v_ag_in = nc.dram_tensor("v_ag_in", [C, H], F32R)
    v_ag = nc.dram_tensor("v_ag", [NC_ * C, H], F32R, addr_space="Shared")
    qrot_d = nc.dram_tensor("qrot_d", [H, C], F32R)

    w_flat = w_ag.rearrange("a b -> (a b)")
    BLK128 = 128 * HCH * 128     # one [128,16,128] block
    BLK256 = 128 * HCH * 256

    def w_block(matrix, idx, bw):
        """Contiguous pre-swizzled [128, HCH, bw] weight block view."""
        base = matrix * H * H + idx * (128 * HCH * bw)
        return w_flat[base: base + 128 * HCH * bw].rearrange(
            "(p c m) -> p c m", p=128, c=HCH)

    hid_v = hid_t.rearrange("a b -> (a b)").rearrange("(p c n) -> p c n", p=128, c=HCH)
    cos_v = cos_s.rearrange("a b -> (a b)").rearrange("(p j s) -> p j s", p=128, j=PAIRS)
    sin_v = sin_s.rearrange("a b -> (a b)").rearrange("(p j s) -> p j s", p=128, j=PAIRS)

    with tile.TileContext(nc) as tc:
        # broadcast the weights before anything else
        nc.sync.dma_start(w_bounce[:], w_sl[:])
        nc.gpsimd.collective_compute(
            "AllGather", mybir.AluOpType.bypass,
            ins=[w_bounce[:]], outs=[w_ag[:]],
            replica_groups=[list(range(NC_))],
        )

        with tc.tile_pool(name="const", bufs=1) as constp:
            iden32 = constp.tile([128, 128], F32)
            make_identity(nc, iden32[:])
            iden_r = constp.tile([128, 128], F32R)
            nc.vector.tensor_copy(iden_r[:], iden32[:])
            iden1 = constp.tile([1, 1], F32)
            nc.vector.memset(iden1[:], 1.0)
            ones32 = constp.tile([128, 1], F32)
            nc.vector.memset(ones32[:], 1.0)
            ones_r = constp.tile([128, 1], F32R)
            nc.vector.tensor_copy(ones_r[:], ones32[:])

            with tc.tile_pool(name="big", bufs=1) as bigp:
                hid_sb = bigp.tile([128, HCH, C], F32R)       # 8 MB, all phases
                nc.sync.dma_start(hid_sb[:], hid_v)

                def projection_phase(wmat, which, cos_sb, sin_sb):
                    """K or Q: project, rope, write k_ag_in/qrot_d (+ krot_o for K)."""
                    with (
                        tc.tile_pool(name=f"wblk_{which}", bufs=3) as wblkp,
                        tc.tile_pool(name=f"kt_{which}", bufs=4) as ktp,
                        tc.tile_pool(name=f"rope_{which}", bufs=2) as ropep,
                        tc.tile_pool(name=f"krot_{which}", bufs=2) as krotp,
                        tc.tile_pool(name=f"ps_{which}", bufs=4, space="PSUM") as psp,
                        tc.tile_pool(name=f"pstr_{which}", bufs=2, space="PSUM") as pstr,
                        tc.tile_pool(name=f"knat_{which}", bufs=3) as knatp,
                    ):
                        dst = k_ag_in if which == "k" else qrot_d
                        for j in range(PAIRS):
                            raws = []
                            for part in (j, j + PAIRS):
                                wb = wblkp.tile([128, HCH, 128], F32R, name="wb", tag="wb")
                                nc.sync.dma_start(wb[:], w_block(wmat, part, 128))
                                raw = ktp.tile([128, C], F32, name="raw", tag="raw")
                                for nchk in range(C // 512):
                                    ps = psp.tile([128, 512], F32, name="ps", tag="ps")
                                    for hch in range(HCH):
                                        nc.tensor.matmul(
                                            ps[:], wb[:, hch, :],
                                            hid_sb[:, hch, nchk * 512:(nchk + 1) * 512],
                                            start=(hch == 0), stop=(hch == HCH - 1),
                                        )
                                    nc.scalar.copy(raw[:, nchk * 512:(nchk + 1) * 512], ps[:])
                                raws.append(raw)
                            re, im = raws
                            t1 = ropep.tile([128, C], F32, name="t1", tag="t1")
                            t2 = ropep.tile([128, C], F32, name="t2", tag="t2")
                            rot_re = krotp.tile([128, C], F32R, name="rot_re", tag="rot_re")
                            rot_im = krotp.tile([128, C], F32R, name="rot_im", tag="rot_im")
                            cj = cos_sb[:, j, None, :].to_broadcast([128, B, SS])
                            sj = sin_sb[:, j, None, :].to_broadcast([128, B, SS])

                            def v3(ap):
                                return ap.rearrange("p (b s) -> p b s", b=B)

                            nc.vector.tensor_mul(v3(t1[:]), v3(re[:]), cj)
                            nc.vector.tensor_mul(v3(t2[:]), v3(im[:]), sj)
                            nc.vector.tensor_tensor(rot_re[:], t1[:], t2[:],
                                                    mybir.AluOpType.subtract)
                            nc.vector.tensor_mul(v3(t1[:]), v3(re[:]), sj)
                            nc.vector.tensor_mul(v3(t2[:]), v3(im[:]), cj)
                            nc.vector.tensor_tensor(rot_im[:], t1[:], t2[:],
                                                    mybir.AluOpType.add)
                            nc.sync.dma_start(dst[j * 128:(j + 1) * 128, :], rot_re[:])
                            nc.sync.dma_start(dst[D2 + j * 128:D2 + (j + 1) * 128, :],
                                              rot_im[:])
                            if which == "k":
                                # natural interleaved k_rot output
                                for sch in range(C // 128):
                                    mini = knatp.tile([128, 256], F32, name="mini", tag="mini")
                                    tpr = pstr.tile([128, 128], F32R, name="tpr", tag="tpr")
                                    nc.tensor.transpose(
                                        tpr[:], rot_re[:, sch * 128:(sch + 1) * 128], iden_r[:])
                                    nc.scalar.copy(mini[:, 0::2], tpr[:])
                                    tpi = pstr.tile([128, 128], F32R, name="tpi", tag="tpi")
                                    nc.tensor.transpose(
                                        tpi[:], rot_im[:, sch * 128:(sch + 1) * 128], iden_r[:])
                                    nc.scalar.copy(mini[:, 1::2], tpi[:])
                                    nc.sync.dma_start(
                                        krot_o[sch * 128:(sch + 1) * 128,
                                               256 * j:256 * (j + 1)],
                                        mini[:])

                with tc.tile_pool(name="cossin", bufs=1) as cosp:
                    cos_sb = cosp.tile([128, PAIRS, SS], F32)
                    sin_sb = cosp.tile([128, PAIRS, SS], F32)
                    nc.sync.dma_start(cos_sb[:], cos_v)
                    nc.sync.dma_start(sin_sb[:], sin_v)

                    projection_phase(1, "k", cos_sb, sin_sb)    # wk
                    nc.gpsimd.collective_compute(
                        "AllGather", mybir.AluOpType.bypass,
                        ins=[k_ag_in[:]], outs=[k_ag[:]],
                        replica_groups=[list(range(NC_))],
                    )
                    projection_phase(0, "q", cos_sb, sin_sb)    # wq

                # ---------------- V projection ----------------
                OG_V = 256
                with (
                    tc.tile_pool(name="vblk", bufs=2) as vblkp,
                    tc.tile_pool(name="v32", bufs=1) as v32p,
                    tc.tile_pool(name="ps_v", bufs=4, space="PSUM") as psvp,
                ):
                    v32s = [v32p.tile([128, H], F32R, name=f"v32_{sch}", tag=f"v32_{sch}")
                            for sch in range(C // 128)]
                    for og in range(H // OG_V):
                        vb = vblkp.tile([128, HCH, OG_V], F32R, name="vb", tag="vb")
                        nc.sync.dma_start(vb[:], w_block(2, og, OG_V))
                        for sch in range(C // 128):
                            ps = psvp.tile([128, OG_V], F32, name="psv", tag="psv")
                            for hch in range(HCH):
                                nc.tensor.matmul(
                                    ps[:], hid_sb[:, hch, sch * 128:(sch + 1) * 128],
                                    vb[:, hch, :],
                                    start=(hch == 0), stop=(hch == HCH - 1),
                                )
                            nc.scalar.copy(v32s[sch][:, og * OG_V:(og + 1) * OG_V], ps[:])
                    for sch in range(C // 128):
                        nc.sync.dma_start(v_ag_in[sch * 128:(sch + 1) * 128, :], v32s[sch][:])
                        nc.sync.dma_start(v_o[sch * 128:(sch + 1) * 128, :], v32s[sch][:])

                nc.gpsimd.collective_compute(
                    "AllGather", mybir.AluOpType.bypass,
                    ins=[v_ag_in[:]], outs=[v_ag[:]],
                    replica_groups=[list(range(NC_))],
                )

            # ---------------- attention ----------------
            KC = S // 128              # 32 context chunks per batch
            with (
                tc.tile_pool(name="qb", bufs=1) as qbp,
                tc.tile_pool(name="kslab", bufs=2) as kslabp,
                tc.tile_pool(name="exps", bufs=1) as expp,
                tc.tile_pool(name="vslab", bufs=4) as vslabp,
                tc.tile_pool(name="ctx", bufs=1) as ctxp,
                tc.tile_pool(name="woblk", bufs=2) as wop,
                tc.tile_pool(name="outs", bufs=2) as outp,
                tc.tile_pool(name="den", bufs=1) as denp,
                tc.tile_pool(name="psmm", bufs=2, space="PSUM") as psmm,
                tc.tile_pool(name="psden", bufs=1, space="PSUM") as psden,
                tc.tile_pool(name="psctx", bufs=1, space="PSUM") as psctx,
            ):
                for b in range(B):
                    qb = qbp.tile([128, HCH, 512], F32R, name="qb", tag="qb")
                    nc.gpsimd.dma_start(
                        qb[:],
                        qrot_d[:, b * 512:(b + 1) * 512].rearrange(
                            "(c p) q -> p c q", p=128))

                    exp_tiles = []
                    den_ps = psden.tile([1, 512], F32, name="den_ps", tag="den_ps")
                    for kc in range(KC):
                        r, l = kc // 4, kc % 4
                        kslab = kslabp.tile([128, HCH, 128], F32R, name="kslab", tag="kslab")
                        k_view = k_ag[r * H:(r + 1) * H,
                                      b * 512 + l * 128: b * 512 + (l + 1) * 128]
                        nc.gpsimd.dma_start(
                            kslab[:], k_view.rearrange("(c p) n -> p c n", p=128))
                        ps_s = psmm.tile([128, 512], F32, name="ps_s", tag="mm")
                        for hch in range(HCH):
                            nc.tensor.matmul(
                                ps_s[:], kslab[:, hch, :], qb[:, hch, :],
                                start=(hch == 0), stop=(hch == HCH - 1),
                            )
                        et = expp.tile([128, 512], F32R, name=f"exp{kc}", tag=f"exp{kc}")
                        nc.scalar.activation(et[:], ps_s[:],
                                             mybir.ActivationFunctionType.Exp,
                                             bias=0.0, scale=SCALE)
                        exp_tiles.append(et)
                        nc.tensor.matmul(den_ps[:], ones_r[:], et[:],
                                         start=(kc == 0), stop=(kc == KC - 1))

                    # denominators -> per-q-row reciprocals [128, 4]
                    den_row = denp.tile([1, 512], F32, name="den_row", tag="den_row")
                    nc.scalar.copy(den_row[:], den_ps[:])
                    den_col = denp.tile([128, 4], F32, name="den_col", tag="den_col")
                    for qs in range(4):
                        tp = psden.tile([128, 1], F32, name="tpd", tag="tpd")
                        nc.tensor.transpose(tp[:], den_row[:, qs * 128:(qs + 1) * 128],
                                            iden1[:])
                        nc.scalar.copy(den_col[:, qs:qs + 1], tp[:])
                    recip = denp.tile([128, 4], F32, name="recip", tag="recip")
                    nc.vector.reciprocal(recip[:], den_col[:])

                    # ctx_t[o, q] = sum_k v[k, o] * numer[k, q]
                    OG_C = 512
                    ctx_tiles = []
                    for og in range(H // OG_C):
                        ps_c = [psctx.tile([128, 512], F32, name=f"psc{os_}", tag=f"psc{os_}")
                                for os_ in range(OG_C // 128)]
                        for kc in range(KC):
                            r, l = kc // 4, kc % 4
                            vslab = vslabp.tile([128, OG_C], F32R, name="vslab", tag="vslab")
                            nc.gpsimd.dma_start(
                                vslab[:],
                                v_ag[r * C + b * 512 + l * 128:
                                     r * C + b * 512 + (l + 1) * 128,
                                     og * OG_C:(og + 1) * OG_C])
                            for os_ in range(OG_C // 128):
                                nc.tensor.matmul(
                                    ps_c[os_][:], vslab[:, os_ * 128:(os_ + 1) * 128],
                                    exp_tiles[kc][:],
                                    start=(kc == 0), stop=(kc == KC - 1),
                                )
                        for os_ in range(OG_C // 128):
                            oc = og * (OG_C // 128) + os_
                            ct = ctxp.tile([128, 512], F32R, name=f"ctx{oc}", tag=f"ctx{oc}")
                            nc.scalar.copy(ct[:], ps_c[os_][:])
                            ctx_tiles.append(ct)

                    # out[q, o'] = (ctx_t.T @ wo_t) * recip[q]
                    OG_O = 256
                    for ogr in range(H // OG_O):
                        wob = wop.tile([128, HCH, OG_O], F32R, name="wob", tag="wob")
                        nc.gpsimd.dma_start(wob[:], w_block(3, ogr, OG_O))
                        for qs in range(4):
                            ps_o = psmm.tile([128, OG_O], F32, name="ps_o", tag="mm")
                            for oc in range(HCH):
                                nc.tensor.matmul(
                                    ps_o[:], ctx_tiles[oc][:, qs * 128:(qs + 1) * 128],
                                    wob[:, oc, :],
                                    start=(oc == 0), stop=(oc == HCH - 1),
                                )
                            ot = outp.tile([128, OG_O], F32, name="ot", tag="ot")
                            nc.vector.tensor_scalar_mul(ot[:], ps_o[:], recip[:, qs:qs + 1])
                            nc.sync.dma_start(
                                out_o[b * 512 + qs * 128: b * 512 + (qs + 1) * 128,
                                      ogr * OG_O:(ogr + 1) * OG_O],
                                ot[:])

    nc.compile()
    return nc


def _get_nc():
    if "nc" not in _NC_CACHE:
        _NC_CACHE["nc"] = build_kernel()
    return _NC_CACHE["nc"]


def _swz(wt, bw):
    """[H, H] -> flat blocks of [128, HCH, bw], contiguous per partition."""
    nb = H // bw
    return np.ascontiguousarray(
        wt.reshape(HCH, 128, nb, bw).transpose(2, 1, 0, 3)).reshape(-1)


def kernel(hidden_states, wq, wk, wv, wo, freqs_cos, freqs_sin, position_ids):
    hidden_states = np.asarray(hidden_states, dtype=np.float32)
    wq = np.asarray(wq, dtype=np.float32)
    wk = np.asarray(wk, dtype=np.float32)
    wv = np.asarray(wv, dtype=np.float32)
    wo = np.asarray(wo, dtype=np.float32)
    pos = np.asarray(position_ids)
    cos = np.asarray(freqs_cos, dtype=np.float32)[pos]   # [S, D2]
    sin = np.asarray(freqs_sin, dtype=np.float32)[pos]

    w_all = np.concatenate([
        _swz(wq.T, 128), _swz(wk.T, 128), _swz(wv.T, 256), _swz(wo.T, 256)])

    in_maps = []
    for i in range(NC_):
        sl = slice(i * SS, (i + 1) * SS)
        hid_i = hidden_states[:, sl, :].transpose(2, 0, 1).reshape(H, C)
        hid_i = np.ascontiguousarray(
            hid_i.reshape(HCH, 128, C).transpose(1, 0, 2)).reshape(H, C)
        cos_i = np.ascontiguousarray(
            cos[sl].T.reshape(PAIRS, 128, SS).transpose(1, 0, 2)).reshape(D2, SS)
        sin_i = np.ascontiguousarray(
            sin[sl].T.reshape(PAIRS, 128, SS).transpose(1, 0, 2)).reshape(D2, SS)
        in_maps.append({
            "hid_t": hid_i,
            "w_sl": w_all[i * WS * H:(i + 1) * WS * H].reshape(WS, H),
            "cos_s": cos_i,
            "sin_s": sin_i,
        })

    nc = _get_nc()
    results = run_bass_kernel_spmd(nc, in_maps, list(range(NC_))).results

    out = np.empty((B, S, H), dtype=np.float32)
    k_rot = np.empty((B, S, H), dtype=np.float32)
    v = np.empty((B, S, H), dtype=np.float32)
    for i in range(NC_):
        sl = slice(i * SS, (i + 1) * SS)
        r = results[i]
        out[:, sl, :] = r["out_o"].reshape(B, SS, H)
        k_rot[:, sl, :] = r["krot_o"].reshape(B, SS, H)
        v[:, sl, :] = r["v_o"].reshape(B, SS, H)
    return out, k_rot, v


# revision 22
# speedup vs baseline: 1.0544x; 1.0112x over previous
"""Trainium2 Bass kernel for nn_CustomAttentionLayer (single-'head' attention
over the full 2048 hidden dim, with module-level RoPE).

Sharding: sequence-parallel over 8 NeuronCores. Each core computes the
q/k/v projections + RoPE for its S/8 = 512 sequence rows (both batches),
exchanges the k_rot/v shards with on-device AllGathers, then runs attention
plus the output projection for its own 512 query rows. The host concatenates
the per-core output shards. The (shared) weights are uploaded sharded 1/8th
per core and broadcast on-device with an AllGather; all weight blocks are
pre-swizzled on the host into [partition, h-chunk, cols] order so each SBUF
weight-tile load is one contiguous 8-16KB descriptor per partition.

Precision: everything runs in float32r (rounded fp32; full PE throughput for
moving dims >= 256) with fp32 PSUM accumulation. Softmax runs unnormalized
(exp without max subtraction -- the fp32 exponent range covers the logit
range) and the per-row normalization is folded in after the output
projection.
"""
import sys
sys.path.insert(0, "/opt/trn_rl_repo")

import numpy as np

from concourse import bacc
import concourse.mybir as mybir
import concourse.tile as tile
from concourse.bass_utils import run_bass_kernel_spmd
from concourse.masks import make_identity

B, S, H = 2, 4096, 2048
NC_ = 8
SS = S // NC_          # 512 sequence rows per core
C = B * SS             # 1024 columns per core (b-major)
D2 = H // 2
SCALE = 1.0 / 8.0
HCH = H // 128         # 16 hidden chunks
PAIRS = D2 // 128      # 8 rope pairs
WS = 4 * H // NC_      # weight-slice rows per core

F32 = mybir.dt.float32
F32R = mybir.dt.float32r

_NC_CACHE = {}


def build_kernel():
    nc = bacc.Bacc("TRN2", target_bir_lowering=False, debug=False, num_devices=NC_)

    # ---- per-core I/O (hid/w/cos pre-swizzled on host, see kernel()) ----
    hid_t = nc.dram_tensor("hid_t", [H, C], F32R, kind="ExternalInput")
    w_sl = nc.dram_tensor("w_sl", [WS, H], F32R, kind="ExternalInput")
    cos_s = nc.dram_tensor("cos_s", [D2, SS], F32, kind="ExternalInput")
    sin_s = nc.dram_tensor("sin_s", [D2, SS], F32, kind="ExternalInput")

    out_o = nc.dram_tensor("out_o", [C, H], F32, kind="ExternalOutput")
    krot_o = nc.dram_tensor("krot_o", [C, H], F32, kind="ExternalOutput")
    v_o = nc.dram_tensor("v_o", [C, H], F32R, kind="ExternalOutput")

    # ---- internal DRAM ----
    w_bounce = nc.dram_tensor("w_bounce", [WS, H], F32R)
    w_ag = nc.dram_tensor("w_ag", [4 * H, H], F32R, addr_space="Shared")
    k_ag_in = nc.dram_tensor("k_ag_in", [H, C], F32R)
    k_ag = nc.dram_tensor("k_ag", [NC_ * H, C], F32R, addr_space="Shared")
    v_ag_in = nc.dram_tensor("v_ag_in", [C, H], F32R)
    v_ag = nc.dram_tensor("v_ag", [NC_ * C, H], F32R, addr_space="Shared")
    qrot_d = nc.dram_tensor("qrot_d", [H, C], F32R)

    w_flat = w_ag.rearrange("a b -> (a b)")

    def w_block(matrix, idx, bw):
        """Contiguous pre-swizzled [128, HCH, bw] weight block view.
        Stacking order in w_ag: wk, wq, wv, wo ('k' == 0)."""
        m = 0 if matrix == "k" else matrix + 1
        base = m * H * H + idx * (128 * HCH * bw)
        return w_flat[base: base + 128 * HCH * bw].rearrange(
            "(p c m) -> p c m", p=128, c=HCH)

    hid_v = hid_t.rearrange("a b -> (a b)").rearrange("(p c n) -> p c n", p=128, c=HCH)
    cos_v = cos_s.rearrange("a b -> (a b)").rearrange("(p j s) -> p j s", p=128, j=PAIRS)
    sin_v = sin_s.rearrange("a b -> (a b)").rearrange("(p j s) -> p j s", p=128, j=PAIRS)

    with tile.TileContext(nc) as tc:
        # broadcast the weights before anything else
        nc.sync.dma_start(w_bounce[:], w_sl[:])
        nc.gpsimd.collective_compute(
            "AllGather", mybir.AluOpType.bypass,
            ins=[w_bounce[:]], outs=[w_ag[:]],
            replica_groups=[list(range(NC_))],
        )

        with tc.tile_pool(name="const", bufs=1) as constp:
            iden32 = constp.tile([128, 128], F32)
            make_identity(nc, iden32[:])
            iden_r = constp.tile([128, 128], F32R)
            nc.vector.tensor_copy(iden_r[:], iden32[:])
            iden1 = constp.tile([1, 1], F32)
            nc.vector.memset(iden1[:], 1.0)
            ones32 = constp.tile([128, 1], F32)
            nc.vector.memset(ones32[:], 1.0)
            ones_r = constp.tile([128, 1], F32R)
            nc.vector.tensor_copy(ones_r[:], ones32[:])

            qbp_cm = tc.tile_pool(name="qb", bufs=1)
            qbp = qbp_cm.__enter__()
            with tc.tile_pool(name="big", bufs=1) as bigp:
                hid_sb = bigp.tile([128, HCH, C], F32R)       # 8 MB, all phases
                nc.sync.dma_start(hid_sb[:], hid_v)

                def projection_phase(wmat, which, cos_sb, sin_sb):
                    """K or Q: project, rope, write k_ag_in/qrot_d (+ krot_o for K)."""
                    with (
                        tc.tile_pool(name=f"wblk_{which}", bufs=3) as wblkp,
                        tc.tile_pool(name=f"kt_{which}", bufs=4) as ktp,
                        tc.tile_pool(name=f"rope_{which}", bufs=2) as ropep,
                        tc.tile_pool(name=f"krot_{which}", bufs=2) as krotp,
                        tc.tile_pool(name=f"ps_{which}", bufs=4, space="PSUM") as psp,
                        tc.tile_pool(name=f"pstr_{which}", bufs=2, space="PSUM") as pstr,
                        tc.tile_pool(name=f"knat_{which}", bufs=3) as knatp,
                    ):
                        dst = k_ag_in if which == "k" else qrot_d
                        for j in range(PAIRS):
                            raws = []
                            for part in (j, j + PAIRS):
                                wb = wblkp.tile([128, HCH, 128], F32R, name="wb", tag="wb")
                                nc.sync.dma_start(wb[:], w_block(wmat, part, 128))
                                raw = ktp.tile([128, C], F32, name="raw", tag="raw")
                                for nchk in range(C // 512):
                                    ps = psp.tile([128, 512], F32, name="ps", tag="ps")
                                    for hch in range(HCH):
                                        nc.tensor.matmul(
                                            ps[:], wb[:, hch, :],
                                            hid_sb[:, hch, nchk * 512:(nchk + 1) * 512],
                                            start=(hch == 0), stop=(hch == HCH - 1),
                                        )
                                    nc.scalar.copy(raw[:, nchk * 512:(nchk + 1) * 512], ps[:])
                                raws.append(raw)
                            re, im = raws
                            t1 = ropep.tile([128, C], F32, name="t1", tag="t1")
                            t2 = ropep.tile([128, C], F32, name="t2", tag="t2")
                            rot_re = krotp.tile([128, C], F32R, name="rot_re", tag="rot_re")
                            rot_im = krotp.tile([128, C], F32R, name="rot_im", tag="rot_im")
                            cj = cos_sb[:, j, None, :].to_broadcast([128, B, SS])
                            sj = sin_sb[:, j, None, :].to_broadcast([128, B, SS])

                            def v3(ap):
                                return ap.rearrange("p (b s) -> p b s", b=B)

                            nc.vector.tensor_mul(v3(t1[:]), v3(re[:]), cj)
                            nc.vector.tensor_mul(v3(t2[:]), v3(im[:]), sj)
                            nc.vector.tensor_tensor(rot_re[:], t1[:], t2[:],
                                                    mybir.AluOpType.subtract)
                            nc.vector.tensor_mul(v3(t1[:]), v3(re[:]), sj)
                            nc.vector.tensor_mul(v3(t2[:]), v3(im[:]), cj)
                            nc.vector.tensor_tensor(rot_im[:], t1[:], t2[:],
                                                    mybir.AluOpType.add)
                            nc.sync.dma_start(dst[j * 128:(j + 1) * 128, :], rot_re[:])
                            nc.sync.dma_start(dst[D2 + j * 128:D2 + (j + 1) * 128, :],
                                              rot_im[:])
                            if which == "k":
                                # natural interleaved k_rot output
                                for sch in range(C // 128):
                                    mini = knatp.tile([128, 256], F32, name="mini", tag="mini")
                                    tpr = pstr.tile([128, 128], F32R, name="tpr", tag="tpr")
                                    nc.tensor.transpose(
                                        tpr[:], rot_re[:, sch * 128:(sch + 1) * 128], iden_r[:])
                                    nc.scalar.copy(mini[:, 0::2], tpr[:])
                                    tpi = pstr.tile([128, 128], F32R, name="tpi", tag="tpi")
                                    nc.tensor.transpose(
                                        tpi[:], rot_im[:, sch * 128:(sch + 1) * 128], iden_r[:])
                                    nc.scalar.copy(mini[:, 1::2], tpi[:])
                                    nc.sync.dma_start(
                                        krot_o[sch * 128:(sch + 1) * 128,
                                               256 * j:256 * (j + 1)],
                                        mini[:])

                with tc.tile_pool(name="cossin", bufs=1) as cosp:
                    cos_sb = cosp.tile([128, PAIRS, SS], F32)
                    sin_sb = cosp.tile([128, PAIRS, SS], F32)
                    nc.sync.dma_start(cos_sb[:], cos_v)
                    nc.sync.dma_start(sin_sb[:], sin_v)

                    projection_phase("k", "k", cos_sb, sin_sb)   # wk
                    nc.gpsimd.collective_compute(
                        "AllGather", mybir.AluOpType.bypass,
                        ins=[k_ag_in[:]], outs=[k_ag[:]],
                        replica_groups=[list(range(NC_))],
                    )
                    projection_phase(0, "q", cos_sb, sin_sb)     # wq

                # pre-stage the b=0 q block before the V phase so its SBUF
                # does not alias freed V-phase tiles (which would chain it
                # behind the V store burst)
                qb0 = qbp.tile([128, HCH, 512], F32R, name="qb", tag="qb")
                nc.scalar.dma_start(
                    qb0[:],
                    qrot_d[:, 0:512].rearrange("(c p) q -> p c q", p=128))

                # ---------------- V projection ----------------
                OG_V = 256
                with (
                    tc.tile_pool(name="vblk", bufs=2) as vblkp,
                    tc.tile_pool(name="v32", bufs=1) as v32p,
                    tc.tile_pool(name="ps_v", bufs=4, space="PSUM") as psvp,
                ):
                    v32s = [v32p.tile([128, H], F32R, name=f"v32_{sch}", tag=f"v32_{sch}")
                            for sch in range(C // 128)]
                    for og in range(H // OG_V):
                        vb = vblkp.tile([128, HCH, OG_V], F32R, name="vb", tag="vb")
                        nc.sync.dma_start(vb[:], w_block(1, og, OG_V))
                        for sch in range(C // 128):
                            ps = psvp.tile([128, OG_V], F32, name="psv", tag="psv")
                            for hch in range(HCH):
                                nc.tensor.matmul(
                                    ps[:], hid_sb[:, hch, sch * 128:(sch + 1) * 128],
                                    vb[:, hch, :],
                                    start=(hch == 0), stop=(hch == HCH - 1),
                                )
                            nc.scalar.copy(v32s[sch][:, og * OG_V:(og + 1) * OG_V], ps[:])
                    for sch in range(C // 128):
                        nc.sync.dma_start(v_ag_in[sch * 128:(sch + 1) * 128, :], v32s[sch][:])
                        nc.sync.dma_start(v_o[sch * 128:(sch + 1) * 128, :], v32s[sch][:])

                nc.gpsimd.collective_compute(
                    "AllGather", mybir.AluOpType.bypass,
                    ins=[v_ag_in[:]], outs=[v_ag[:]],
                    replica_groups=[list(range(NC_))],
                )

            # ---------------- attention ----------------
            KC = S // 128              # 32 context chunks per batch
            with (
                tc.tile_pool(name="kslab", bufs=2) as kslabp,
                tc.tile_pool(name="exps", bufs=1) as expp,
                tc.tile_pool(name="vslab", bufs=4) as vslabp,
                tc.tile_pool(name="ctx", bufs=1) as ctxp,
                tc.tile_pool(name="woblk", bufs=2) as wop,
                tc.tile_pool(name="outs", bufs=2) as outp,
                tc.tile_pool(name="den", bufs=1) as denp,
                tc.tile_pool(name="psmm", bufs=2, space="PSUM") as psmm,
                tc.tile_pool(name="psden", bufs=1, space="PSUM") as psden,
                tc.tile_pool(name="psctx", bufs=1, space="PSUM") as psctx,
            ):
                for b in range(B):
                    if b == 0:
                        qb = qb0
                    else:
                        qb = qbp.tile([128, HCH, 512], F32R, name="qb", tag="qb")
                        nc.scalar.dma_start(
                            qb[:],
                            qrot_d[:, b * 512:(b + 1) * 512].rearrange(
                                "(c p) q -> p c q", p=128))

                    exp_tiles = []
                    den_ps = psden.tile([1, 512], F32, name="den_ps", tag="den_ps")
                    for kc2 in range(KC // 2):
                        r, l2 = kc2 // 2, kc2 % 2
                        kslab = kslabp.tile([128, HCH, 256], F32R, name="kslab", tag="kslab")
                        k_view = k_ag[r * H:(r + 1) * H,
                                      b * 512 + l2 * 256: b * 512 + (l2 + 1) * 256]
                        nc.scalar.dma_start(
                            kslab[:], k_view.rearrange("(c p) n -> p c n", p=128))
                        for half in range(2):
                            kc = kc2 * 2 + half
                            ps_s = psmm.tile([128, 512], F32, name="ps_s", tag="mm")
                            for hch in range(HCH):
                                nc.tensor.matmul(
                                    ps_s[:],
                                    kslab[:, hch, half * 128:(half + 1) * 128],
                                    qb[:, hch, :],
                                    start=(hch == 0), stop=(hch == HCH - 1),
                                )
                            et = expp.tile([128, 512], F32R, name=f"exp{kc}", tag=f"exp{kc}")
                            nc.scalar.activation(et[:], ps_s[:],
                                                 mybir.ActivationFunctionType.Exp,
                                                 bias=0.0, scale=SCALE)
                            exp_tiles.append(et)
                            nc.tensor.matmul(den_ps[:], ones_r[:], et[:],
                                             start=(kc == 0), stop=(kc == KC - 1))

                    # denominators -> per-q-row reciprocals [128, 4]
                    den_row = denp.tile([1, 512], F32, name="den_row", tag="den_row")
                    nc.scalar.copy(den_row[:], den_ps[:])
                    den_col = denp.tile([128, 4], F32, name="den_col", tag="den_col")
                    for qs in range(4):
                        tp = psden.tile([128, 1], F32, name="tpd", tag="tpd")
                        nc.tensor.transpose(tp[:], den_row[:, qs * 128:(qs + 1) * 128],
                                            iden1[:])
                        nc.scalar.copy(den_col[:, qs:qs + 1], tp[:])
                    recip = denp.tile([128, 4], F32, name="recip", tag="recip")
                    nc.vector.reciprocal(recip[:], den_col[:])

                    # ctx_t[o, q] = sum_k v[k, o] * numer[k, q]
                    OG_C = 512
                    ctx_tiles = []
                    for og in range(H // OG_C):
                        ps_c = [psctx.tile([128, 512], F32, name=f"psc{os_}", tag=f"psc{os_}")
                                for os_ in range(OG_C // 128)]
                        for kc in range(KC):
                            r, l = kc // 4, kc % 4
                            vslab = vslabp.tile([128, OG_C], F32R, name="vslab", tag="vslab")
                            nc.gpsimd.dma_start(
                                vslab[:],
                                v_ag[r * C + b * 512 + l * 128:
                                     r * C + b * 512 + (l + 1) * 128,
                                     og * OG_C:(og + 1) * OG_C])
                            for os_ in range(OG_C // 128):
                                nc.tensor.matmul(
                                    ps_c[os_][:], vslab[:, os_ * 128:(os_ + 1) * 128],
                                    exp_tiles[kc][:],
                                    start=(kc == 0), stop=(kc == KC - 1),
                                )
                        for os_ in range(OG_C // 128):
                            oc = og * (OG_C // 128) + os_
                            ct = ctxp.tile([128, 512], F32R, name=f"ctx{oc}", tag=f"ctx{oc}")
                            nc.scalar.copy(ct[:], ps_c[os_][:])
                            ctx_tiles.append(ct)

                    # out[q, o'] = (ctx_t.T @ wo_t) * recip[q]
                    OG_O = 256
                    for ogr in range(H // OG_O):
                        wob = wop.tile([128, HCH, OG_O], F32R, name="wob", tag="wob")
                        nc.gpsimd.dma_start(wob[:], w_block(2, ogr, OG_O))
                        for qs in range(4):
                            ps_o = psmm.tile([128, OG_O], F32, name="ps_o", tag="mm")
                            for oc in range(HCH):
                                nc.tensor.matmul(
                                    ps_o[:], ctx_tiles[oc][:, qs * 128:(qs + 1) * 128],
                                    wob[:, oc, :],
                                    start=(oc == 0), stop=(oc == HCH - 1),
                                )
                            ot = outp.tile([128, OG_O], F32, name="ot", tag="ot")
                            nc.vector.tensor_scalar_mul(ot[:], ps_o[:], recip[:, qs:qs + 1])
                            nc.sync.dma_start(
                                out_o[b * 512 + qs * 128: b * 512 + (qs + 1) * 128,
                                      ogr * OG_O:(ogr + 1) * OG_O],
                                ot[:])
            qbp_cm.__exit__(None, None, None)

    nc.compile()
    return nc


def _get_nc():
    if "nc" not in _NC_CACHE:
        _NC_CACHE["nc"] = build_kernel()
    return _NC_CACHE["nc"]


def _swz(wt, bw):
    """[H, H] -> flat blocks of [128, HCH, bw], contiguous per partition."""
    nb = H // bw
    return np.ascontiguousarray(
        wt.reshape(HCH, 128, nb, bw).transpose(2, 1, 0, 3)).reshape(-1)


def kernel(hidden_states, wq, wk, wv, wo, freqs_cos, freqs_sin, position_ids):
    hidden_states = np.asarray(hidden_states, dtype=np.float32)
    wq = np.asarray(wq, dtype=np.float32)
    wk = np.asarray(wk, dtype=np.float32)
    wv = np.asarray(wv, dtype=np.float32)
    wo = np.asarray(wo, dtype=np.float32)
    pos = np.asarray(position_ids)
    cos = np.asarray(freqs_cos, dtype=np.float32)[pos]   # [S, D2]
    sin = np.asarray(freqs_sin, dtype=np.float32)[pos]

    w_all = np.concatenate([
        _swz(wk.T, 128), _swz(wq.T, 128), _swz(wv.T, 256), _swz(wo.T, 256)])

    in_maps = []
    for i in range(NC_):
        sl = slice(i * SS, (i + 1) * SS)
        hid_i = hidden_states[:, sl, :].transpose(2, 0, 1).reshape(H, C)
        hid_i = np.ascontiguousarray(
            hid_i.reshape(HCH, 128, C).transpose(1, 0, 2)).reshape(H, C)
        cos_i = np.ascontiguousarray(
            cos[sl].T.reshape(PAIRS, 128, SS).transpose(1, 0, 2)).reshape(D2, SS)
        sin_i = np.ascontiguousarray(
            sin[sl].T.reshape(PAIRS, 128, SS).transpose(1, 0, 2)).reshape(D2, SS)
        in_maps.append({
            "hid_t": hid_i,
            "w_sl": w_all[i * WS * H:(i + 1) * WS * H].reshape(WS, H),
            "cos_s": cos_i,
            "sin_s": sin_i,
        })

    nc = _get_nc()
    results = run_bass_kernel_spmd(nc, in_maps, list(range(NC_))).results

    out = np.empty((B, S, H), dtype=np.float32)
    k_rot = np.empty((B, S, H), dtype=np.float32)
    v = np.empty((B, S, H), dtype=np.float32)
    for i in range(NC_):
        sl = slice(i * SS, (i + 1) * SS)
        r = results[i]
        out[:, sl, :] = r["out_o"].reshape(B, SS, H)
        k_rot[:, sl, :] = r["krot_o"].reshape(B, SS, H)
        v[:, sl, :] = r["v_o"].reshape(B, SS, H)
    return out, k_rot, v
